# revision 27
# baseline (speedup 1.0000x reference)
"""Trainium2 Bass kernel for CustomCombinedLoss (weighted BCE sum + MultiMarginLoss).

loss = -sum(w * (pos_t*log(p) + (1-pos_t)*log(1-p)))          # w=2 for target==0
     + sum_{i: target_i>0} (1/C) * sum_{j != y_i} max(0, margin - x[i,y_i] + x[i,j])

Sharding: pure data parallel over the batch dim, B=16384 rows -> 8 cores x 2048 rows.
Each core computes a partial scalar loss; host sums the 8 partials.

Host-side layout tricks (index/dtype transforms only):
  - predictions cast to bf16 (loss rel-err ~2e-6, far under the 2e-2 tolerance):
    halves the HBM->SBUF traffic, the roofline here (435 GB/s DMA fabric per core).
  - per row, pred[r, y_r] is swapped with pred[r, 0].  Row hinge sums are
    permutation-invariant so the math is unchanged, and x[y] is always column 0:
    no one-hot/iota extraction pass is needed on device.
  - x[y] (column 0) is also duplicated into a small aux tensor with tgt/pprob, so
    all 16 hinge bias columns are ready before any pred tile lands.
  - row groups 1..14 are interleaved pairwise on the host so each pair is one
    contiguous [128, 4096] DMA (1 MB transfers run nearer the fabric ceiling and
    halve the trigger count); group 0 is DMA'd as two halves to start compute
    early, group 15 alone so the tail tile lands as early as possible.

Per-core device program (rows on partitions, C on the free axis):
  - hinge row-sums sum_j relu(pred + (margin - x[y])) are computed whole-tile per
    engine -- ACT (Relu activation + accum) and DVE (STT add/max + accum) each own
    one group of every pair, so per-tile fixed costs (accumulator readout) are
    paid once.  Group 0 is split across both engines by half, group 15 by columns,
    to shorten pipeline head and tail.  The j==y self-term contributes exactly
    relu(margin) = margin, subtracted in the epilogue.
  - BCE terms on [128,16] tiles: r = pos ? p : 1-p, one Ln on ACT,
    row_total = (acc_a + acc_d)*pos/C - (pos*margin/C + (2-pos)*max(ln r, -100)).
  - free-axis reduce on DVE, cross-partition reduce via PE matmul -> scalar out.
"""

from contextlib import ExitStack

import numpy as np
import ml_dtypes

import concourse.bacc as bacc
import concourse.bass as bass
import concourse.mybir as mybir
import concourse.tile as tile
from concourse.bass_utils import run_bass_kernel_spmd

WEIGHT = 2.0
MARGIN = 0.5
B, C = 16384, 2048
NCORES = 8
BS = B // NCORES          # rows per core
P = 128                   # partitions
T = BS // P               # row tiles (groups) per core
NPAIR = (T - 2) // 2      # paired groups 1..14
F32 = mybir.dt.float32

AluOp = mybir.AluOpType
ActFn = mybir.ActivationFunctionType
AxisList = mybir.AxisListType

# Knobs
PRED_DT = mybir.dt.float8e4          # pred dtype on device
PRED_NP = ml_dtypes.float8_e4m3      # matching numpy dtype
SPLIT_A = 832                        # ACT columns of split groups (DVE gets the rest)


def _loss_program(nc: bass.Bass, tc: "tile.TileContext", pred, predp, aux, out):
    ctx = ExitStack()
    with ctx:
        small_pool = ctx.enter_context(tc.tile_pool(name="small", bufs=1))
        pred_pool = ctx.enter_context(tc.tile_pool(name="pred", bufs=1))

        # aux = [tgt | pprob | xy] as [P, 3T] f32 on ACT's HWDGE queue, in
        # parallel with the first pred triggers on Sync
        aux_t = small_pool.tile([P, 3 * T], F32)
        nc.scalar.dma_start(aux_t[:], aux[:])
        tgt_t = aux_t[:, 0:T]
        pprob_t = aux_t[:, T : 2 * T]
        xy_t = aux_t[:, 2 * T : 3 * T]

        # pred DMAs on the Sync queue.  Group 0 in quarters (earliest engine
        # start during the slow DMA ramp), groups 1-3 singles, 4..13 as 1 MB
        # pairs (best sustained fabric rate), 14-15 singles (short tail).
        Q = C // 4
        quarters = []
        for q in range(4):
            pq = pred_pool.tile([P, Q], PRED_DT, tag=f"q{q}")
            nc.sync.dma_start(pq[:], pred[0:P, q * Q : (q + 1) * Q])
            quarters.append(pq)
        group_tiles = {}
        for g in (1, 2, 3):
            ptile = pred_pool.tile([P, C], PRED_DT, tag="head", bufs=3)
            nc.sync.dma_start(ptile[:], pred[bass.ts(g, P), :])
            group_tiles[g] = ptile
        pair_tiles = []
        for k in range(4):
            ptile = pred_pool.tile([P, 2 * C], PRED_DT, tag="pair", bufs=4)
            nc.sync.dma_start(ptile[:], predp[k])
            pair_tiles.append(ptile)
        for g in (12, 13, 14, 15):
            ptile = pred_pool.tile([P, C], PRED_DT, tag="tail", bufs=4)
            nc.sync.dma_start(ptile[:], pred[bass.ts(g, P), :])
            group_tiles[g] = ptile

        # scratch outputs (never read) + shared zeros for the STT in1 operand
        junk_a = small_pool.tile([P, C], PRED_DT)
        junk_d = small_pool.tile([P, C], PRED_DT)
        zeros_t = small_pool.tile([P, C], PRED_DT)
        nc.gpsimd.memset(zeros_t[:], 0.0)

        # per-group accumulators; each engine writes only its groups' columns.
        # column T holds the second-quarter partials of group 0.
        acc_a = small_pool.tile([P, T + 1], F32)
        nc.vector.memset(acc_a[:], 0.0)
        acc_d = small_pool.tile([P, T + 1], F32)
        nc.vector.memset(acc_d[:], 0.0)

        # hinge bias columns for all groups: bias = margin - x[y]  (DVE, one op)
        bias_all = small_pool.tile([P, T], F32)
        nc.vector.tensor_scalar(
            bias_all[:], xy_t, -1.0, MARGIN, AluOp.mult, AluOp.add
        )

        # ---- BCE-side preprocessing (overlaps the pred DMA stream) ----
        # pos = min(tgt,1); r = pos ? p : 1-p = pos*(2p-1) + (1-p)
        pos_t = small_pool.tile([P, T], F32)
        nc.vector.tensor_scalar(pos_t[:], tgt_t, 1.0, None, AluOp.min)
        t1 = small_pool.tile([P, T], F32)      # 1 - p
        nc.vector.tensor_scalar(t1[:], pprob_t, -1.0, 1.0, AluOp.mult, AluOp.add)
        t2 = small_pool.tile([P, T], F32)      # 2p - 1
        nc.vector.tensor_scalar(t2[:], pprob_t, 2.0, -1.0, AluOp.mult, AluOp.add)
        m_t = small_pool.tile([P, T], F32)
        nc.vector.tensor_mul(m_t[:], t2[:], pos_t[:])
        r_t = small_pool.tile([P, T], F32)
        nc.vector.tensor_add(r_t[:], m_t[:], t1[:])
        lr_t = small_pool.tile([P, T], F32)
        nc.scalar.activation(lr_t[:], r_t[:], ActFn.Ln)
        nc.vector.tensor_scalar(lr_t[:], lr_t[:], -100.0, None, AluOp.max)
        w_t = small_pool.tile([P, T], F32)     # 2 - pos
        nc.vector.tensor_scalar(w_t[:], pos_t[:], -1.0, 2.0, AluOp.mult, AluOp.add)
        d_t = small_pool.tile([P, T], F32)     # (2-pos)*lr
        nc.vector.tensor_mul(d_t[:], w_t[:], lr_t[:])
        u_t = small_pool.tile([P, T], F32)     # pos/C
        nc.vector.tensor_scalar(u_t[:], pos_t[:], 1.0 / C, None, AluOp.mult)
        v_t = small_pool.tile([P, T], F32)     # pos*margin/C + d
        nc.vector.scalar_tensor_tensor(
            v_t[:], pos_t[:], MARGIN / C, d_t[:], AluOp.mult, AluOp.add
        )

        # ---- hinge body ----
        def act_hinge(src_ap, bias_g, acc_col, width=C):
            nc.scalar.activation(
                junk_a[:, 0:width], src_ap, ActFn.Relu,
                bias=bias_all[:, bias_g : bias_g + 1], scale=1.0,
                accum_out=acc_a[:, acc_col : acc_col + 1],
            )

        def dve_hinge(src_ap, bias_g, acc_col, width=C):
            nc.vector.scalar_tensor_tensor(
                junk_d[:, 0:width], src_ap, bias_all[:, bias_g : bias_g + 1],
                zeros_t[:, 0:width], AluOp.add, AluOp.max,
                accum_out=acc_d[:, acc_col : acc_col + 1],
            )

        # group 0: quarters alternating ACT/DVE (second pair lands in column T)
        act_hinge(quarters[0][:], 0, 0, Q)
        dve_hinge(quarters[1][:], 0, 0, Q)
        act_hinge(quarters[2][:], 0, T, Q)
        dve_hinge(quarters[3][:], 0, T, Q)
        # fold group-0's second-quarter partials (column T) into column 0 now,
        # off the critical tail
        x0 = small_pool.tile([P, 1], F32)
        nc.vector.tensor_add(x0[:], acc_a[:, T : T + 1], acc_d[:, T : T + 1])
        nc.vector.tensor_add(acc_a[:, 0:1], acc_a[:, 0:1], x0[:])
        # head singles
        act_hinge(group_tiles[1][:], 1, 1)
        dve_hinge(group_tiles[2][:], 2, 2)
        act_hinge(group_tiles[3][:], 3, 3)
        # pairs (4+2k, 5+2k): ACT takes the first group, DVE the second
        for k in range(4):
            ptile = pair_tiles[k]
            act_hinge(ptile[:, 0:C], 4 + 2 * k, 4 + 2 * k)
            dve_hinge(ptile[:, C : 2 * C], 5 + 2 * k, 5 + 2 * k)
        # tail: groups 12..15 column-split across both engines so the last
        # 2 MB is consumed at the combined-engine rate
        A = SPLIT_A
        for g in (12, 13, 14, 15):
            pt = group_tiles[g]
            act_hinge(pt[:, 0:A], g, g, A)
            dve_hinge(pt[:, A:C], g, g, C - A)

        # ---- epilogue:  e = (acc_a + acc_d)*u - v;  total = sum(e) ----
        s_t = small_pool.tile([P, T], F32)
        nc.vector.tensor_add(s_t[:], acc_a[:, 0:T], acc_d[:, 0:T])
        m2 = small_pool.tile([P, T], F32)
        nc.vector.tensor_mul(m2[:], s_t[:], u_t[:])
        e_t = small_pool.tile([P, T], F32)
        nc.vector.tensor_sub(e_t[:], m2[:], v_t[:])
        rowred = small_pool.tile([P, 1], F32)
        nc.vector.reduce_sum(rowred[:], e_t[:], axis=AxisList.X)
        nc.sync.dma_start(out[:], rowred[:])


def build_nc() -> bass.Bass:
    nc = bacc.Bacc("TRN2", target_bir_lowering=False, debug=False, num_devices=NCORES)
    pred = nc.dram_tensor("pred", [BS, C], PRED_DT, kind="ExternalInput").ap()
    predp = nc.dram_tensor("predp", [4, P, 2 * C], PRED_DT, kind="ExternalInput").ap()
    aux = nc.dram_tensor("aux", [P, 3 * T], F32, kind="ExternalInput").ap()
    out = nc.dram_tensor("out", [P, 1], F32, kind="ExternalOutput").ap()
    with tile.TileContext(nc) as tc:
        _loss_program(nc, tc, pred, predp, aux, out)
    nc.compile()
    return nc


def make_in_maps(positive_prob, predictions, target):
    """Shard full inputs into per-core input maps (host-side layout only)."""
    in_maps = []
    idx = np.arange(BS)
    for i in range(NCORES):
        sl = slice(i * BS, (i + 1) * BS)
        pp = np.asarray(positive_prob[sl], dtype=np.float32)
        tg64 = np.asarray(target[sl]).astype(np.int64)
        pr = np.asarray(predictions[sl], dtype=np.float32).astype(PRED_NP)
        # swap pred[r, y_r] <-> pred[r, 0] so x[y] sits at a fixed column;
        # row sums are permutation-invariant so the loss is unchanged.
        y = np.maximum(tg64 - 1, 0)
        a = pr[idx, y].copy()
        b = pr[idx, 0].copy()
        pr[idx, 0] = a
        pr[idx, y] = b
        # [BS] -> [P, T]: row g*P + p lands at [p, g], matching the row tiling
        def toPT(v):
            return np.asarray(v, dtype=np.float32).reshape(T, P).T
        aux = np.ascontiguousarray(
            np.concatenate(
                [toPT(tg64), toPT(pp), toPT(a.astype(np.float32))], axis=1
            )
        )
        # pairs cover groups 4..11: interleave so pair k is one [128, 4096] DMA
        pairs = pr[4 * P : 12 * P].reshape(4, 2, P, C)
        predp = np.ascontiguousarray(pairs.transpose(0, 2, 1, 3).reshape(4, P, 2 * C))
        in_maps.append({"pred": np.ascontiguousarray(pr), "predp": predp, "aux": aux})
    return in_maps


_NC_CACHE = []


def kernel(positive_prob, predictions, target):
    in_maps = make_in_maps(positive_prob, predictions, target)
    if not _NC_CACHE:
        _NC_CACHE.append(build_nc())
    nc = _NC_CACHE[0]
    res = run_bass_kernel_spmd(nc, in_maps, list(range(NCORES)))
    total = np.float32(0.0)
    for r in res.results:
        total += np.asarray(r["out"], dtype=np.float32).sum(dtype=np.float32)
    return np.asarray(total, dtype=np.float32)


# revision 28
# speedup vs baseline: 1.1740x; 1.1740x over previous
"""Trainium2 Bass kernel for CustomCombinedLoss (weighted BCE sum + MultiMarginLoss).

loss = -sum(w * (pos_t*log(p) + (1-pos_t)*log(1-p)))          # w=2 for target==0
     + sum_{i: target_i>0} (1/C) * sum_{j != y_i} max(0, margin - x[i,y_i] + x[i,j])

Sharding: pure data parallel over the batch dim, B=16384 rows -> 8 cores x 2048 rows.
Each core computes a partial scalar loss; host sums the 8 partials.

Host-side layout tricks (index/dtype transforms only):
  - predictions cast to bf16 (loss rel-err ~2e-6, far under the 2e-2 tolerance):
    halves the HBM->SBUF traffic, the roofline here (435 GB/s DMA fabric per core).
  - per row, pred[r, y_r] is swapped with pred[r, 0].  Row hinge sums are
    permutation-invariant so the math is unchanged, and x[y] is always column 0:
    no one-hot/iota extraction pass is needed on device.
  - x[y] (column 0) is also duplicated into a small aux tensor with tgt/pprob, so
    all 16 hinge bias columns are ready before any pred tile lands.
  - row groups 1..14 are interleaved pairwise on the host so each pair is one
    contiguous [128, 4096] DMA (1 MB transfers run nearer the fabric ceiling and
    halve the trigger count); group 0 is DMA'd as two halves to start compute
    early, group 15 alone so the tail tile lands as early as possible.

Per-core device program (rows on partitions, C on the free axis):
  - hinge row-sums sum_j relu(pred + (margin - x[y])) are computed whole-tile per
    engine -- ACT (Relu activation + accum) and DVE (STT add/max + accum) each own
    one group of every pair, so per-tile fixed costs (accumulator readout) are
    paid once.  Group 0 is split across both engines by half, group 15 by columns,
    to shorten pipeline head and tail.  The j==y self-term contributes exactly
    relu(margin) = margin, subtracted in the epilogue.
  - BCE terms on [128,16] tiles: r = pos ? p : 1-p, one Ln on ACT,
    row_total = (acc_a + acc_d)*pos/C - (pos*margin/C + (2-pos)*max(ln r, -100)).
  - free-axis reduce on DVE, cross-partition reduce via PE matmul -> scalar out.
"""

from contextlib import ExitStack

import numpy as np
import ml_dtypes

import concourse.bacc as bacc
import concourse.bass as bass
import concourse.mybir as mybir
import concourse.tile as tile
from concourse.bass_utils import run_bass_kernel_spmd

WEIGHT = 2.0
MARGIN = 0.5
B, C = 16384, 2048
NCORES = 8
BS = B // NCORES          # rows per core
P = 128                   # partitions
T = BS // P               # row tiles (groups) per core
NPAIR = (T - 2) // 2      # paired groups 1..14
F32 = mybir.dt.float32

AluOp = mybir.AluOpType
ActFn = mybir.ActivationFunctionType
AxisList = mybir.AxisListType

# Knobs
PRED_DT = mybir.dt.float8e4          # pred dtype on device
PRED_NP = ml_dtypes.float8_e4m3      # matching numpy dtype
SPLIT_A = 832                        # ACT columns of split groups (DVE gets the rest)


def _loss_program(nc: bass.Bass, tc: "tile.TileContext", pred, predp, aux, out):
    ctx = ExitStack()
    with ctx:
        small_pool = ctx.enter_context(tc.tile_pool(name="small", bufs=1))
        pred_pool = ctx.enter_context(tc.tile_pool(name="pred", bufs=1))

        # aux = [tgt | pprob | xy] as [P, 3T] f32 on ACT's HWDGE queue, in
        # parallel with the first pred triggers on Sync
        aux_t = small_pool.tile([P, 3 * T], F32)
        nc.scalar.dma_start(aux_t[:], aux[:])
        tgt_t = aux_t[:, 0:T]
        pprob_t = aux_t[:, T : 2 * T]
        xy_t = aux_t[:, 2 * T : 3 * T]

        # pred DMAs on the Sync queue.  Group 0 in quarters (earliest engine
        # start during the slow DMA ramp), groups 1-3 singles, 4..13 as 1 MB
        # pairs (best sustained fabric rate), 14-15 singles (short tail).
        Q = C // 4
        quarters = []
        for q in range(4):
            pq = pred_pool.tile([P, Q], PRED_DT, tag=f"q{q}")
            nc.sync.dma_start(pq[:], pred[0:P, q * Q : (q + 1) * Q])
            quarters.append(pq)
        group_tiles = {}
        for g in (1, 2, 3):
            ptile = pred_pool.tile([P, C], PRED_DT, tag="head", bufs=3)
            nc.sync.dma_start(ptile[:], pred[bass.ts(g, P), :])
            group_tiles[g] = ptile
        pair_tiles = []
        for k in range(4):
            ptile = pred_pool.tile([P, 2 * C], PRED_DT, tag="pair", bufs=4)
            nc.sync.dma_start(ptile[:], predp[k])
            pair_tiles.append(ptile)
        for g in (12, 13, 14, 15):
            ptile = pred_pool.tile([P, C], PRED_DT, tag="tail", bufs=4)
            nc.sync.dma_start(ptile[:], pred[bass.ts(g, P), :])
            group_tiles[g] = ptile

        # scratch outputs (never read) + shared zeros for the STT in1 operand
        junk_a = small_pool.tile([P, C], PRED_DT)
        junk_d = small_pool.tile([P, C], PRED_DT)
        zeros_t = small_pool.tile([P, C], PRED_DT)
        nc.gpsimd.memset(zeros_t[:], 0.0)

        ones_t = small_pool.tile([P, 1], F32)
        nc.vector.memset(ones_t[:], 1.0)

        # per-group accumulators; each engine writes only its groups' columns.
        # column T holds the second-quarter partials of group 0.
        acc_a = small_pool.tile([P, T + 1], F32)
        nc.vector.memset(acc_a[:], 0.0)
        acc_d = small_pool.tile([P, T + 1], F32)
        nc.vector.memset(acc_d[:], 0.0)

        # hinge bias columns for all groups: bias = margin - x[y]  (DVE, one op)
        bias_all = small_pool.tile([P, T], F32)
        nc.vector.tensor_scalar(
            bias_all[:], xy_t, -1.0, MARGIN, AluOp.mult, AluOp.add
        )

        # ---- BCE-side preprocessing (overlaps the pred DMA stream) ----
        # pos = min(tgt,1); r = pos ? p : 1-p = pos*(2p-1) + (1-p)
        pos_t = small_pool.tile([P, T], F32)
        nc.vector.tensor_scalar(pos_t[:], tgt_t, 1.0, None, AluOp.min)
        t1 = small_pool.tile([P, T], F32)      # 1 - p
        nc.vector.tensor_scalar(t1[:], pprob_t, -1.0, 1.0, AluOp.mult, AluOp.add)
        t2 = small_pool.tile([P, T], F32)      # 2p - 1
        nc.vector.tensor_scalar(t2[:], pprob_t, 2.0, -1.0, AluOp.mult, AluOp.add)
        m_t = small_pool.tile([P, T], F32)
        nc.vector.tensor_mul(m_t[:], t2[:], pos_t[:])
        r_t = small_pool.tile([P, T], F32)
        nc.vector.tensor_add(r_t[:], m_t[:], t1[:])
        lr_t = small_pool.tile([P, T], F32)
        nc.scalar.activation(lr_t[:], r_t[:], ActFn.Ln)
        nc.vector.tensor_scalar(lr_t[:], lr_t[:], -100.0, None, AluOp.max)
        w_t = small_pool.tile([P, T], F32)     # 2 - pos
        nc.vector.tensor_scalar(w_t[:], pos_t[:], -1.0, 2.0, AluOp.mult, AluOp.add)
        d_t = small_pool.tile([P, T], F32)     # (2-pos)*lr
        nc.vector.tensor_mul(d_t[:], w_t[:], lr_t[:])
        u_t = small_pool.tile([P, T], F32)     # pos/C
        nc.vector.tensor_scalar(u_t[:], pos_t[:], 1.0 / C, None, AluOp.mult)
        v_t = small_pool.tile([P, T], F32)     # pos*margin/C + d
        nc.vector.scalar_tensor_tensor(
            v_t[:], pos_t[:], MARGIN / C, d_t[:], AluOp.mult, AluOp.add
        )

        # ---- hinge body ----
        def act_hinge(src_ap, bias_g, acc_col, width=C):
            nc.scalar.activation(
                junk_a[:, 0:width], src_ap, ActFn.Relu,
                bias=bias_all[:, bias_g : bias_g + 1], scale=1.0,
                accum_out=acc_a[:, acc_col : acc_col + 1],
            )

        def dve_hinge(src_ap, bias_g, acc_col, width=C):
            nc.vector.scalar_tensor_tensor(
                junk_d[:, 0:width], src_ap, bias_all[:, bias_g : bias_g + 1],
                zeros_t[:, 0:width], AluOp.add, AluOp.max,
                accum_out=acc_d[:, acc_col : acc_col + 1],
            )

        # group 0: quarters alternating ACT/DVE (second pair lands in column T)
        act_hinge(quarters[0][:], 0, 0, Q)
        dve_hinge(quarters[1][:], 0, 0, Q)
        act_hinge(quarters[2][:], 0, T, Q)
        dve_hinge(quarters[3][:], 0, T, Q)
        # fold group-0's second-quarter partials (column T) into column 0 now,
        # off the critical tail
        x0 = small_pool.tile([P, 1], F32)
        nc.vector.tensor_add(x0[:], acc_a[:, T : T + 1], acc_d[:, T : T + 1])
        nc.vector.tensor_add(acc_a[:, 0:1], acc_a[:, 0:1], x0[:])
        # head singles
        act_hinge(group_tiles[1][:], 1, 1)
        dve_hinge(group_tiles[2][:], 2, 2)
        act_hinge(group_tiles[3][:], 3, 3)
        # pairs (4+2k, 5+2k): ACT takes the first group, DVE the second
        for k in range(4):
            ptile = pair_tiles[k]
            act_hinge(ptile[:, 0:C], 4 + 2 * k, 4 + 2 * k)
            dve_hinge(ptile[:, C : 2 * C], 5 + 2 * k, 5 + 2 * k)
        # tail: groups 12..15 column-split across both engines so the last
        # 2 MB is consumed at the combined-engine rate
        A = SPLIT_A
        for g in (12, 13, 14, 15):
            pt = group_tiles[g]
            act_hinge(pt[:, 0:A], g, g, A)
            dve_hinge(pt[:, A:C], g, g, C - A)

        # ---- epilogue:  e = (acc_a + acc_d)*u - v;  total = sum(e) ----
        s_t = small_pool.tile([P, T], F32)
        nc.vector.tensor_add(s_t[:], acc_a[:, 0:T], acc_d[:, 0:T])
        m2 = small_pool.tile([P, T], F32)
        nc.vector.tensor_mul(m2[:], s_t[:], u_t[:])
        e_t = small_pool.tile([P, T], F32)
        nc.vector.tensor_sub(e_t[:], m2[:], v_t[:])
        rowred = small_pool.tile([P, 1], F32)
        nc.vector.reduce_sum(rowred[:], e_t[:], axis=AxisList.X)
        psum_pool = ctx.enter_context(tc.tile_pool(name="psum", bufs=1, space="PSUM"))
        total_ps = psum_pool.tile([1, 1], F32)
        nc.tensor.matmul(total_ps[:], rowred[:], ones_t[:], start=True, stop=True)
        total = small_pool.tile([1, 1], F32)
        nc.vector.tensor_copy(total[:], total_ps[:])
        nc.sync.dma_start(out[:], total[:])


def build_nc() -> bass.Bass:
    nc = bacc.Bacc("TRN2", target_bir_lowering=False, debug=False, num_devices=NCORES)
    pred = nc.dram_tensor("pred", [BS, C], PRED_DT, kind="ExternalInput").ap()
    predp = nc.dram_tensor("predp", [4, P, 2 * C], PRED_DT, kind="ExternalInput").ap()
    aux = nc.dram_tensor("aux", [P, 3 * T], F32, kind="ExternalInput").ap()
    out = nc.dram_tensor("out", [1, 1], F32, kind="ExternalOutput").ap()
    with tile.TileContext(nc) as tc:
        _loss_program(nc, tc, pred, predp, aux, out)
    nc.compile()
    return nc


def make_in_maps(positive_prob, predictions, target):
    """Shard full inputs into per-core input maps (host-side layout only)."""
    in_maps = []
    idx = np.arange(BS)
    for i in range(NCORES):
        sl = slice(i * BS, (i + 1) * BS)
        pp = np.asarray(positive_prob[sl], dtype=np.float32)
        tg64 = np.asarray(target[sl]).astype(np.int64)
        pr = np.asarray(predictions[sl], dtype=np.float32).astype(PRED_NP)
        # swap pred[r, y_r] <-> pred[r, 0] so x[y] sits at a fixed column;
        # row sums are permutation-invariant so the loss is unchanged.
        y = np.maximum(tg64 - 1, 0)
        a = pr[idx, y].copy()
        b = pr[idx, 0].copy()
        pr[idx, 0] = a
        pr[idx, y] = b
        # [BS] -> [P, T]: row g*P + p lands at [p, g], matching the row tiling
        def toPT(v):
            return np.asarray(v, dtype=np.float32).reshape(T, P).T
        aux = np.ascontiguousarray(
            np.concatenate(
                [toPT(tg64), toPT(pp), toPT(a.astype(np.float32))], axis=1
            )
        )
        # pairs cover groups 4..11: interleave so pair k is one [128, 4096] DMA
        pairs = pr[4 * P : 12 * P].reshape(4, 2, P, C)
        predp = np.ascontiguousarray(pairs.transpose(0, 2, 1, 3).reshape(4, P, 2 * C))
        in_maps.append({"pred": np.ascontiguousarray(pr), "predp": predp, "aux": aux})
    return in_maps


_NC_CACHE = []


def kernel(positive_prob, predictions, target):
    in_maps = make_in_maps(positive_prob, predictions, target)
    if not _NC_CACHE:
        _NC_CACHE.append(build_nc())
    nc = _NC_CACHE[0]
    res = run_bass_kernel_spmd(nc, in_maps, list(range(NCORES)))
    total = np.float32(0.0)
    for r in res.results:
        total += np.asarray(r["out"], dtype=np.float32).sum(dtype=np.float32)
    return np.asarray(total, dtype=np.float32)
